# revision 1
# baseline (speedup 1.0000x reference)
"""Trainium2 Bass kernel for nn_MemoryCore (retrieval KNN min-distance).

Problem: embedding [8192, 512], memory_bank [65536, 512] (fp32) ->
patch_scores [8192, 1] = min over the bank of euclidean distance.

Strategy (8 NeuronCores, SPMD):
  - Shard the memory bank (M axis) 8 ways; every core sees all queries.
  - Per core: psum[m, n] = (-2*bank_shard) @ emb.T via PE (float32r,
    1 cyc/row), fused running min over m-tiles on DVE via
    scalar_tensor_tensor: RM = min(psum + m_sq[m], RM).
  - Epilogue per 512-query block: PE-transpose RM, reduce_min over the
    free axis, sqrt(min + x_sq) on ACT -> per-core local min distances.
  - Host: elementwise min across the 8 cores.
"""
import numpy as np
import concourse.bacc as bacc
import concourse.mybir as mybir
import concourse.tile as tile
from concourse.bass_utils import run_bass_kernel_spmd
from concourse.masks import make_identity

N_CORES = 8
N, M, D = 8192, 65536, 512
MS = M // N_CORES       # 8192 bank rows per core
MSB = 1024              # bank chunk width (columns) per persistent tile
PSUM_BUFS = 6
BIG = 1e30
DT = mybir.dt.float32r  # TF32-like matmul: 4x faster than fp32, ~1e-4 rel err

_CACHE = {}


def _build_kernel():
    K = D // 128            # contraction chunks
    NB = N // 512           # query blocks (free axis)
    MT = MS // 128          # bank tiles (partitions)
    NMSB = MS // MSB
    mt_per_chunk = MSB // 128

    nc = bacc.Bacc("TRN2", target_bir_lowering=False, debug=False,
                   num_devices=N_CORES)

    embT_d = nc.dram_tensor("embT", [D, N], DT, kind="ExternalInput")
    bankT_d = nc.dram_tensor("bankT", [D, MS], DT, kind="ExternalInput")
    msq_d = nc.dram_tensor("msq", [128, MT], mybir.dt.float32, kind="ExternalInput")
    xsq_d = nc.dram_tensor("xsq", [128, N // 128], mybir.dt.float32, kind="ExternalInput")
    out_d = nc.dram_tensor("out", [128, N // 128], mybir.dt.float32, kind="ExternalOutput")

    with tile.TileContext(nc) as tc:
        with (
            tc.tile_pool(name="persist", bufs=1) as persist,
            tc.tile_pool(name="emb", bufs=2) as embp,
            tc.tile_pool(name="rmp", bufs=2) as rmp,
            tc.tile_pool(name="small", bufs=4) as small,
            tc.tile_pool(name="psum", bufs=PSUM_BUFS, space="PSUM") as psum,
            tc.tile_pool(name="psum_t", bufs=2, space="PSUM") as psum_t,
        ):
            msq = persist.tile([128, MT], mybir.dt.float32, tag="msq")
            nc.gpsimd.dma_start(msq[:], msq_d[:])
            xsq = persist.tile([128, N // 128], mybir.dt.float32, tag="xsq")
            nc.gpsimd.dma_start(xsq[:], xsq_d[:])
            out_s = persist.tile([128, N // 128], mybir.dt.float32, tag="outs")
            ident = persist.tile([128, 128], mybir.dt.float32, tag="ident")
            make_identity(nc, ident)

            def load_emb(nb):
                t = embp.tile([128, K, 512], DT, tag="embt")
                for k in range(K):
                    nc.gpsimd.dma_start(
                        t[:, k, :],
                        embT_d[k * 128:(k + 1) * 128, nb * 512:(nb + 1) * 512])
                return t

            emb_next = load_emb(0)

            bank_t = [[None] * NMSB for _ in range(K)]
            for j in range(NMSB):
                for k in range(K):
                    t = persist.tile([128, MSB], DT, tag=f"bank{k}_{j}")
                    nc.sync.dma_start(
                        t[:], bankT_d[k * 128:(k + 1) * 128, j * MSB:(j + 1) * MSB])
                    bank_t[k][j] = t

            for nb in range(NB):
                emb_t = emb_next
                if nb + 1 < NB:
                    emb_next = load_emb(nb + 1)
                rm = rmp.tile([128, 512], mybir.dt.float32, tag="rm")
                nc.vector.memset(rm[:], BIG)
                for mt in range(MT):
                    j, jj = mt // mt_per_chunk, mt % mt_per_chunk
                    ps = psum.tile([128, 512], mybir.dt.float32, tag="ps")
                    for k in range(K):
                        nc.tensor.matmul(
                            ps[:],
                            bank_t[k][j][:, jj * 128:(jj + 1) * 128],
                            emb_t[:, k, :],
                            start=(k == 0),
                            stop=(k == K - 1),
                        )
                    # RM = min(psum + m_sq[m], RM)  (one DVE op, reads PSUM)
                    nc.vector.scalar_tensor_tensor(
                        out=rm[:],
                        in0=ps[:],
                        scalar=msq[:, mt:mt + 1],
                        in1=rm[:],
                        op0=mybir.AluOpType.add,
                        op1=mybir.AluOpType.min,
                    )
                for q in range(4):  # cross-partition min per 128-query chunk
                    pt = psum_t.tile([128, 128], mybir.dt.float32, tag="pt")
                    nc.tensor.transpose(pt[:], rm[:, q * 128:(q + 1) * 128], ident[:])
                    mn = small.tile([128, 1], mybir.dt.float32, tag="mn")
                    nc.vector.tensor_reduce(
                        out=mn[:], in_=pt[:], axis=mybir.AxisListType.X,
                        op=mybir.AluOpType.min)
                    col = nb * 4 + q
                    nc.scalar.activation(
                        out=out_s[:, col:col + 1],
                        in_=mn[:],
                        func=mybir.ActivationFunctionType.Sqrt,
                        bias=xsq[:, col:col + 1],
                        scale=1.0,
                    )
            nc.sync.dma_start(out_d[:], out_s[:])

    nc.compile()
    return nc


def kernel(embedding: np.ndarray, memory_bank: np.ndarray) -> np.ndarray:
    emb = np.asarray(embedding, dtype=np.float32)
    bank = np.asarray(memory_bank, dtype=np.float32)
    assert emb.shape == (N, D) and bank.shape == (M, D)

    if "nc" not in _CACHE:
        _CACHE["nc"] = _build_kernel()
    nc = _CACHE["nc"]

    embT = np.ascontiguousarray(emb.T)
    x_sq = np.einsum("nd,nd->n", emb, emb, dtype=np.float64).astype(np.float32)
    xsq = np.ascontiguousarray(x_sq.reshape(N // 128, 128).T)

    in_maps = []
    for c in range(N_CORES):
        shard = bank[c * MS:(c + 1) * MS]
        bankT = np.ascontiguousarray((-2.0 * shard).T)
        m_sq = np.einsum("md,md->m", shard, shard, dtype=np.float64).astype(np.float32)
        msq = np.ascontiguousarray(m_sq.reshape(MS // 128, 128).T)
        in_maps.append({"embT": embT, "bankT": bankT, "msq": msq, "xsq": xsq})

    _CACHE["last_in_maps"] = in_maps
    try:
        res = run_bass_kernel_spmd(nc, in_maps, core_ids=list(range(N_CORES)))
    except Exception:
        # a previously-wedged NeuronCore reports unrecoverable once and then
        # recovers; one retry clears it
        import time
        time.sleep(2.0)
        res = run_bass_kernel_spmd(nc, in_maps, core_ids=list(range(N_CORES)))

    # gather: each core returns [128, N/128] local min distances; min over cores
    per_core = np.stack([res.results[c]["out"].T.reshape(N) for c in range(N_CORES)])
    return per_core.min(axis=0).reshape(N, 1).astype(np.float32)



# revision 2
# speedup vs baseline: 1.3873x; 1.3873x over previous
"""Trainium2 Bass kernel for nn_MemoryCore (retrieval KNN min-distance).

Problem: embedding [8192, 512], memory_bank [65536, 512] (fp32) ->
patch_scores [8192, 1] = min over the bank of euclidean distance.

Strategy (8 NeuronCores, SPMD):
  - Shard the memory bank (M axis) 8 ways; every core sees all queries.
  - fp8(E4M3) inputs, PE DoubleRow matmuls (256-deep contraction per MM,
    2 MMs per psum tile instead of 4): psum[m, n] = (-2*bank) @ emb.
  - Min stage split across DVE and ACT so neither is the bottleneck:
      DVE route: rm = min(psum + m_sq[m], rm)   (one fused STT, bf16 rm)
      ACT route: t = Identity(psum + m_sq[m]) -> bf16 SBUF (scalar engine),
                 rm = min(t, rm)                (2-byte DVE op, 4x mode)
  - Epilogue per 512-query block: PE-transpose rm (bf16), reduce_min,
    then one final add(x_sq) + sqrt at the end.
  - Host: elementwise min across the 8 cores.
"""
import numpy as np
import ml_dtypes
import concourse.bacc as bacc
import concourse.mybir as mybir
import concourse.tile as tile
from concourse.bass_utils import run_bass_kernel_spmd
from concourse.masks import make_identity

N_CORES = 8
N, M, D = 8192, 65536, 512
MS = M // N_CORES       # 8192 bank rows per core
K4 = D // 128           # 4 fp8 contraction planes of 128
NB = N // 512           # 16 query blocks
MT = MS // 128          # 64 bank tiles
BIG = 1e30
DT8 = mybir.dt.float8e4
NP8 = ml_dtypes.float8_e4m3
DVE_ROUTE = 3           # of every 8 bank tiles, this many go to the DVE
                        # fused-STT route; the rest go ACT drain + bf16 min

_CACHE = {}


def _build_kernel():
    nc = bacc.Bacc("TRN2", target_bir_lowering=False, debug=False,
                   num_devices=N_CORES)

    embT_d = nc.dram_tensor("embT8", [128, K4 * N], DT8, kind="ExternalInput")
    bankT_d = nc.dram_tensor("bankT8", [128, K4 * MS], DT8, kind="ExternalInput")
    msq_d = nc.dram_tensor("msq", [128, MT], mybir.dt.float32, kind="ExternalInput")
    xsq_d = nc.dram_tensor("xsq", [128, N // 128], mybir.dt.float32, kind="ExternalInput")
    out_d = nc.dram_tensor("out", [128, N // 128], mybir.dt.float32, kind="ExternalOutput")

    DR = mybir.MatmulPerfMode.DoubleRow

    with tile.TileContext(nc) as tc:
        with (
            tc.tile_pool(name="persist", bufs=1) as persist,
            tc.tile_pool(name="rmp", bufs=2) as rmp,
            tc.tile_pool(name="tp", bufs=6) as tp,
            tc.tile_pool(name="small", bufs=4) as small,
            tc.tile_pool(name="psum", bufs=6, space="PSUM") as psum,
            tc.tile_pool(name="psum_t", bufs=2, space="PSUM") as psum_t,
        ):
            msq = persist.tile([128, MT], mybir.dt.float32, tag="msq")
            nc.gpsimd.dma_start(msq[:], msq_d[:])
            xsq = persist.tile([128, N // 128], mybir.dt.float32, tag="xsq")
            nc.gpsimd.dma_start(xsq[:], xsq_d[:])
            mins_s = persist.tile([128, N // 128], mybir.dt.float32, tag="mins")
            out_s = persist.tile([128, N // 128], mybir.dt.float32, tag="outs")
            ident = persist.tile([128, 128], mybir.dt.bfloat16, tag="ident")
            make_identity(nc, ident)

            emb_s = persist.tile([128, K4, N], DT8, tag="embs")
            bank_s = persist.tile([128, K4, MS], DT8, tag="banks")
            for k in range(K4):
                nc.sync.dma_start(emb_s[:, k, :], embT_d[:, k * N:(k + 1) * N])
                nc.sync.dma_start(bank_s[:, k, :], bankT_d[:, k * MS:(k + 1) * MS])

            for nb in range(NB):
                rm = rmp.tile([128, 512], mybir.dt.bfloat16, tag="rm")
                nc.vector.memset(rm[:], BIG)
                for mt in range(MT):
                    ps = psum.tile([128, 512], mybir.dt.float32, tag="ps")
                    for kk in range(2):
                        nc.tensor.matmul(
                            ps[:],
                            bank_s[:, 2 * kk:2 * kk + 2, mt * 128:(mt + 1) * 128],
                            emb_s[:, 2 * kk:2 * kk + 2, nb * 512:(nb + 1) * 512],
                            start=(kk == 0),
                            stop=(kk == 1),
                            perf_mode=DR,
                        )
                    if mt % 8 < DVE_ROUTE:
                        # DVE: rm = min(psum + m_sq[m], rm) in one op
                        nc.vector.scalar_tensor_tensor(
                            out=rm[:],
                            in0=ps[:],
                            scalar=msq[:, mt:mt + 1],
                            in1=rm[:],
                            op0=mybir.AluOpType.add,
                            op1=mybir.AluOpType.min,
                        )
                    else:
                        # ACT drains psum (+m_sq bias) to bf16, DVE mins it
                        t = tp.tile([128, 512], mybir.dt.bfloat16, tag="t")
                        nc.scalar.add(t[:], ps[:], msq[:, mt:mt + 1])
                        nc.vector.tensor_tensor(
                            out=rm[:], in0=t[:], in1=rm[:],
                            op=mybir.AluOpType.min)
                for q in range(4):  # cross-partition min per 128-query chunk
                    pt = psum_t.tile([128, 128], mybir.dt.bfloat16, tag="pt")
                    nc.tensor.transpose(pt[:], rm[:, q * 128:(q + 1) * 128], ident[:])
                    col = nb * 4 + q
                    nc.vector.tensor_reduce(
                        out=mins_s[:, col:col + 1], in_=pt[:],
                        axis=mybir.AxisListType.X, op=mybir.AluOpType.min)
            # dist = sqrt(min + x_sq), one op each over the full [128, 64]
            nc.vector.tensor_tensor(
                out=mins_s[:], in0=mins_s[:], in1=xsq[:],
                op=mybir.AluOpType.add)
            nc.scalar.sqrt(out_s[:], mins_s[:])
            nc.sync.dma_start(out_d[:], out_s[:])

    nc.compile()
    return nc


def _pack_kT(mat_T: np.ndarray, width: int) -> np.ndarray:
    """[D, width] fp32 -> [128, K4, width] fp8 with plane k = rows k*128..+128."""
    return np.ascontiguousarray(
        mat_T.reshape(K4, 128, width).transpose(1, 0, 2)).astype(NP8)


def kernel(embedding: np.ndarray, memory_bank: np.ndarray) -> np.ndarray:
    emb = np.asarray(embedding, dtype=np.float32)
    bank = np.asarray(memory_bank, dtype=np.float32)
    assert emb.shape == (N, D) and bank.shape == (M, D)

    if "nc" not in _CACHE:
        _CACHE["nc"] = _build_kernel()
    nc = _CACHE["nc"]

    embT8 = _pack_kT(emb.T, N).reshape(128, K4 * N)
    x_sq = np.einsum("nd,nd->n", emb, emb, dtype=np.float64).astype(np.float32)
    xsq = np.ascontiguousarray(x_sq.reshape(N // 128, 128).T)

    in_maps = []
    for c in range(N_CORES):
        shard = bank[c * MS:(c + 1) * MS]
        bankT8 = _pack_kT((-2.0 * shard).T, MS).reshape(128, K4 * MS)
        m_sq = np.einsum("md,md->m", shard, shard, dtype=np.float64).astype(np.float32)
        msq = np.ascontiguousarray(m_sq.reshape(MS // 128, 128).T)
        in_maps.append({"embT8": embT8, "bankT8": bankT8, "msq": msq, "xsq": xsq})

    _CACHE["last_in_maps"] = in_maps
    try:
        res = run_bass_kernel_spmd(nc, in_maps, core_ids=list(range(N_CORES)))
    except Exception:
        # a previously-wedged NeuronCore reports unrecoverable once and then
        # recovers; one retry clears it
        import time
        time.sleep(2.0)
        res = run_bass_kernel_spmd(nc, in_maps, core_ids=list(range(N_CORES)))

    # gather: each core returns [128, N/128] local min distances; min over cores
    per_core = np.stack([res.results[c]["out"].T.reshape(N) for c in range(N_CORES)])
    return per_core.min(axis=0).reshape(N, 1).astype(np.float32)


# revision 5
# speedup vs baseline: 1.6038x; 1.1560x over previous
"""Trainium2 Bass kernel for nn_MemoryCore (retrieval KNN min-distance).

Problem: embedding [8192, 512], memory_bank [65536, 512] (fp32) ->
patch_scores [8192, 1] = min over the bank of euclidean distance.

Strategy (8 NeuronCores, SPMD):
  - Shard the memory bank (M axis) 8 ways; every core sees all queries.
  - fp8(E4M3) inputs, PE DoubleRow matmuls (256-deep contraction per MM,
    2 MMs per psum tile instead of 4): psum[m, n] = (-2*bank) @ emb.
  - Min stage split across DVE and ACT so neither is the bottleneck:
      DVE route: rm = min(psum + m_sq[m], rm)   (one fused STT, bf16 rm)
      ACT route: t = Identity(psum + m_sq[m]) -> bf16 SBUF (scalar engine),
                 rm = min(t, rm)                (2-byte DVE op, 4x mode)
  - Epilogue per 512-query block: PE-transpose rm (bf16), reduce_min,
    then one final add(x_sq) + sqrt at the end.
  - Host: elementwise min across the 8 cores.
"""
import numpy as np
import ml_dtypes
import concourse.bacc as bacc
import concourse.mybir as mybir
import concourse.tile as tile
from concourse.bass_utils import run_bass_kernel_spmd
from concourse.masks import make_identity

N_CORES = 8
N, M, D = 8192, 65536, 512
MS = M // N_CORES       # 8192 bank rows per core
K4 = D // 128           # 4 fp8 contraction planes of 128
QB = 1024               # query block width (psum tile spans 2 banks)
NB = N // QB            # 8 query blocks
MT = MS // 128          # 64 bank tiles
BIG = 1e30
DT8 = mybir.dt.float8e4
NP8 = ml_dtypes.float8_e4m3
DVE_ROUTE = 2           # of every 8 bank tiles, this many go to the DVE
                        # fused-STT route; the rest go ACT drain + bf16 min

_CACHE = {}


def _build_kernel():
    nc = bacc.Bacc("TRN2", target_bir_lowering=False, debug=False,
                   num_devices=N_CORES)

    embT_d = nc.dram_tensor("embT8", [128, K4 * N], DT8, kind="ExternalInput")
    bankT_d = nc.dram_tensor("bankT8", [128, K4 * MS], DT8, kind="ExternalInput")
    msq_d = nc.dram_tensor("msq", [128, MT], mybir.dt.float32, kind="ExternalInput")
    xsq_d = nc.dram_tensor("xsq", [128, N // 128], mybir.dt.float32, kind="ExternalInput")
    out_d = nc.dram_tensor("out", [128, N // 128], mybir.dt.float32, kind="ExternalOutput")

    DR = mybir.MatmulPerfMode.DoubleRow

    with tile.TileContext(nc) as tc:
        with (
            tc.tile_pool(name="persist", bufs=1) as persist,
            tc.tile_pool(name="rmp", bufs=2) as rmp,
            tc.tile_pool(name="tp", bufs=4) as tp,
            tc.tile_pool(name="psum", bufs=3, space="PSUM") as psum,
            tc.tile_pool(name="psum_t", bufs=2, space="PSUM") as psum_t,
        ):
            msq = persist.tile([128, MT], mybir.dt.float32, tag="msq")
            nc.gpsimd.dma_start(msq[:], msq_d[:])
            xsq = persist.tile([128, N // 128], mybir.dt.float32, tag="xsq")
            nc.gpsimd.dma_start(xsq[:], xsq_d[:])
            mins_s = persist.tile([128, N // 128], mybir.dt.float32, tag="mins")
            out_s = persist.tile([128, N // 128], mybir.dt.float32, tag="outs")
            ident = persist.tile([128, 128], mybir.dt.bfloat16, tag="ident")
            make_identity(nc, ident)

            emb_s = persist.tile([128, K4, N], DT8, tag="embs")
            bank_s = persist.tile([128, K4, MS], DT8, tag="banks")
            for k in range(K4):
                nc.sync.dma_start(emb_s[:, k, :], embT_d[:, k * N:(k + 1) * N])
                nc.sync.dma_start(bank_s[:, k, :], bankT_d[:, k * MS:(k + 1) * MS])

            for nb in range(NB):
                rm = rmp.tile([128, QB], mybir.dt.bfloat16, tag="rm")
                nc.vector.memset(rm[:], BIG)
                for mt in range(MT):
                    ps = psum.tile([128, QB], mybir.dt.float32, tag="ps")
                    for kk in range(2):  # each 512-col half is one PSUM bank
                        for h in range(2):
                            nc.tensor.matmul(
                                ps[:, h * 512:(h + 1) * 512],
                                bank_s[:, 2 * kk:2 * kk + 2, mt * 128:(mt + 1) * 128],
                                emb_s[:, 2 * kk:2 * kk + 2,
                                      nb * QB + h * 512:nb * QB + (h + 1) * 512],
                                start=(kk == 0),
                                stop=(kk == 1),
                                perf_mode=DR,
                            )
                    if mt % 8 < DVE_ROUTE:
                        # DVE: rm = min(psum + m_sq[m], rm) in one op
                        nc.vector.scalar_tensor_tensor(
                            out=rm[:],
                            in0=ps[:],
                            scalar=msq[:, mt:mt + 1],
                            in1=rm[:],
                            op0=mybir.AluOpType.add,
                            op1=mybir.AluOpType.min,
                        )
                    else:
                        # ACT drains psum (+m_sq bias) to bf16, DVE mins it
                        t = tp.tile([128, QB], mybir.dt.bfloat16, tag="t")
                        nc.scalar.add(t[:], ps[:], msq[:, mt:mt + 1])
                        nc.vector.tensor_tensor(
                            out=rm[:], in0=t[:], in1=rm[:],
                            op=mybir.AluOpType.min)
                for q in range(QB // 128):  # cross-partition min per 128 queries
                    pt = psum_t.tile([128, 128], mybir.dt.bfloat16, tag="pt")
                    nc.tensor.transpose(pt[:], rm[:, q * 128:(q + 1) * 128], ident[:])
                    col = nb * (QB // 128) + q
                    nc.vector.tensor_reduce(
                        out=mins_s[:, col:col + 1], in_=pt[:],
                        axis=mybir.AxisListType.X, op=mybir.AluOpType.min)
            # dist = sqrt(min + x_sq), one op each over the full [128, 64]
            nc.vector.tensor_tensor(
                out=mins_s[:], in0=mins_s[:], in1=xsq[:],
                op=mybir.AluOpType.add)
            nc.scalar.sqrt(out_s[:], mins_s[:])
            nc.sync.dma_start(out_d[:], out_s[:])

    nc.compile()
    return nc


def _pack_kT(mat_T: np.ndarray, width: int) -> np.ndarray:
    """[D, width] fp32 -> [128, K4, width] fp8 with plane k = rows k*128..+128."""
    return np.ascontiguousarray(
        mat_T.reshape(K4, 128, width).transpose(1, 0, 2)).astype(NP8)


def kernel(embedding: np.ndarray, memory_bank: np.ndarray) -> np.ndarray:
    emb = np.asarray(embedding, dtype=np.float32)
    bank = np.asarray(memory_bank, dtype=np.float32)
    assert emb.shape == (N, D) and bank.shape == (M, D)

    if "nc" not in _CACHE:
        _CACHE["nc"] = _build_kernel()
    nc = _CACHE["nc"]

    embT8 = _pack_kT(emb.T, N).reshape(128, K4 * N)
    x_sq = np.einsum("nd,nd->n", emb, emb, dtype=np.float64).astype(np.float32)
    xsq = np.ascontiguousarray(x_sq.reshape(N // 128, 128).T)

    in_maps = []
    for c in range(N_CORES):
        shard = bank[c * MS:(c + 1) * MS]
        bankT8 = _pack_kT((-2.0 * shard).T, MS).reshape(128, K4 * MS)
        m_sq = np.einsum("md,md->m", shard, shard, dtype=np.float64).astype(np.float32)
        msq = np.ascontiguousarray(m_sq.reshape(MS // 128, 128).T)
        in_maps.append({"embT8": embT8, "bankT8": bankT8, "msq": msq, "xsq": xsq})

    _CACHE["last_in_maps"] = in_maps
    try:
        res = run_bass_kernel_spmd(nc, in_maps, core_ids=list(range(N_CORES)))
    except Exception:
        # a previously-wedged NeuronCore reports unrecoverable once and then
        # recovers; one retry clears it
        import time
        time.sleep(2.0)
        res = run_bass_kernel_spmd(nc, in_maps, core_ids=list(range(N_CORES)))

    # gather: each core returns [128, N/128] local min distances; min over cores
    per_core = np.stack([res.results[c]["out"].T.reshape(N) for c in range(N_CORES)])
    return per_core.min(axis=0).reshape(N, 1).astype(np.float32)


# revision 6
# speedup vs baseline: 1.9983x; 1.2460x over previous
"""Trainium2 Bass kernel for nn_MemoryCore (retrieval KNN min-distance).

Problem: embedding [8192, 512], memory_bank [65536, 512] (fp32) ->
patch_scores [8192, 1] = min over the bank of euclidean distance.

Strategy (8 NeuronCores, SPMD):
  - Shard the memory bank (M axis) 8 ways; every core sees all queries.
  - fp8(E4M3) inputs, PE DoubleRow matmuls (256-deep contraction per MM):
    psum[m, q] = (-2*bank) @ emb, psum tiles span 2 banks (1024 queries).
  - Min stage split across DVE and ACT so neither is the bottleneck:
      DVE route: rm = min(psum + m_sq[m], rm)   (one fused STT, bf16 rm)
      ACT route: t = Identity(psum + m_sq[m]) -> bf16 SBUF (scalar engine),
                 rm = min(t, rm)                (2-byte DVE op)
    Two rm chains (even/odd bank tile) decouple the DVE dependency chain.
  - Per-core result: rm [128, 8192] bf16 (128 bank slots x all queries),
    DMA'd to HBM. Host does the cross-partition min, +x_sq, sqrt, and the
    min across the 8 cores.
"""
import numpy as np
import ml_dtypes
import concourse.bacc as bacc
import concourse.mybir as mybir
import concourse.tile as tile
from concourse.bass_utils import run_bass_kernel_spmd

N_CORES = 8
N, M, D = 8192, 65536, 512
MS = M // N_CORES       # 8192 bank rows per core
K4 = D // 128           # 4 fp8 contraction planes of 128
QB = 1024               # query block width (psum tile spans 2 banks)
NB = N // QB            # 8 query blocks
MT = MS // 128          # 64 bank tiles
BIG = 1e30
DT8 = mybir.dt.float8e4
NP8 = ml_dtypes.float8_e4m3

_CACHE = {}


def _build_kernel():
    nc = bacc.Bacc("TRN2", target_bir_lowering=False, debug=False,
                   num_devices=N_CORES)

    embT_d = nc.dram_tensor("embT8", [128, K4 * N], DT8, kind="ExternalInput")
    bankT_d = nc.dram_tensor("bankT8", [128, K4 * MS], DT8, kind="ExternalInput")
    msq_d = nc.dram_tensor("msq", [128, MT], mybir.dt.float32, kind="ExternalInput")
    rm_d = nc.dram_tensor("rm_out", [128, N], mybir.dt.bfloat16,
                          kind="ExternalOutput")

    DR = mybir.MatmulPerfMode.DoubleRow

    with tile.TileContext(nc) as tc:
        with (
            tc.tile_pool(name="persist", bufs=1) as persist,
            tc.tile_pool(name="rmp", bufs=4) as rmp,
            tc.tile_pool(name="tp", bufs=4) as tp,
            tc.tile_pool(name="psum", bufs=4, space="PSUM") as psum,
        ):
            msq = persist.tile([128, MT], mybir.dt.float32, tag="msq")
            nc.gpsimd.dma_start(msq[:], msq_d[:])

            emb_s = persist.tile([128, K4, N], DT8, tag="embs")
            bank_s = persist.tile([128, K4, MS], DT8, tag="banks")
            # first query block + first bank tiles land first so the MM
            # stream starts ~2us in instead of ~8us
            for k in range(K4):
                nc.sync.dma_start(emb_s[:, k, :QB], embT_d[:, k * N:k * N + QB])
            for k in range(K4):
                nc.gpsimd.dma_start(bank_s[:, k, :QB],
                                    bankT_d[:, k * MS:k * MS + QB])
            for k in range(K4):
                nc.sync.dma_start(emb_s[:, k, QB:], embT_d[:, k * N + QB:(k + 1) * N])
            for k in range(K4):
                nc.gpsimd.dma_start(bank_s[:, k, QB:],
                                    bankT_d[:, k * MS + QB:(k + 1) * MS])

            for nb in range(NB):
                rm_a = rmp.tile([128, QB], mybir.dt.bfloat16, tag="rma")
                rm_b = rmp.tile([128, QB], mybir.dt.bfloat16, tag="rmb")
                nc.vector.memset(rm_a[:], BIG)
                nc.vector.memset(rm_b[:], BIG)
                for mt in range(MT):
                    rm = rm_a if mt % 2 == 0 else rm_b
                    ps = psum.tile([128, QB], mybir.dt.float32, tag="ps")
                    for kk in range(2):  # each 512-col half is one PSUM bank
                        for h in range(2):
                            nc.tensor.matmul(
                                ps[:, h * 512:(h + 1) * 512],
                                bank_s[:, 2 * kk:2 * kk + 2, mt * 128:(mt + 1) * 128],
                                emb_s[:, 2 * kk:2 * kk + 2,
                                      nb * QB + h * 512:nb * QB + (h + 1) * 512],
                                start=(kk == 0),
                                stop=(kk == 1),
                                perf_mode=DR,
                            )
                    if mt % 4 == 0:
                        # DVE: rm = min(psum + m_sq[m], rm) in one op
                        nc.vector.scalar_tensor_tensor(
                            out=rm[:],
                            in0=ps[:],
                            scalar=msq[:, mt:mt + 1],
                            in1=rm[:],
                            op0=mybir.AluOpType.add,
                            op1=mybir.AluOpType.min,
                        )
                    else:
                        # ACT drains psum (+m_sq bias) to bf16, DVE mins it
                        t = tp.tile([128, QB], mybir.dt.bfloat16, tag="t")
                        nc.scalar.add(t[:], ps[:], msq[:, mt:mt + 1])
                        nc.vector.tensor_tensor(
                            out=rm[:], in0=t[:], in1=rm[:],
                            op=mybir.AluOpType.min)
                nc.vector.tensor_tensor(
                    out=rm_a[:], in0=rm_b[:], in1=rm_a[:],
                    op=mybir.AluOpType.min)
                nc.sync.dma_start(rm_d[:, nb * QB:(nb + 1) * QB], rm_a[:])

    nc.compile()
    return nc


def _pack_kT(mat_T: np.ndarray, width: int) -> np.ndarray:
    """[D, width] fp32 -> [128, K4, width] fp8 with plane k = rows k*128..+128."""
    return np.ascontiguousarray(
        mat_T.reshape(K4, 128, width).transpose(1, 0, 2)).astype(NP8)


def kernel(embedding: np.ndarray, memory_bank: np.ndarray) -> np.ndarray:
    emb = np.asarray(embedding, dtype=np.float32)
    bank = np.asarray(memory_bank, dtype=np.float32)
    assert emb.shape == (N, D) and bank.shape == (M, D)

    if "nc" not in _CACHE:
        _CACHE["nc"] = _build_kernel()
    nc = _CACHE["nc"]

    embT8 = _pack_kT(emb.T, N).reshape(128, K4 * N)
    x_sq = np.einsum("nd,nd->n", emb, emb, dtype=np.float64).astype(np.float32)

    in_maps = []
    for c in range(N_CORES):
        shard = bank[c * MS:(c + 1) * MS]
        bankT8 = _pack_kT((-2.0 * shard).T, MS).reshape(128, K4 * MS)
        m_sq = np.einsum("md,md->m", shard, shard, dtype=np.float64).astype(np.float32)
        msq = np.ascontiguousarray(m_sq.reshape(MS // 128, 128).T)
        in_maps.append({"embT8": embT8, "bankT8": bankT8, "msq": msq})

    _CACHE["last_in_maps"] = in_maps
    try:
        res = run_bass_kernel_spmd(nc, in_maps, core_ids=list(range(N_CORES)))
    except Exception:
        # a previously-wedged NeuronCore reports unrecoverable once and then
        # recovers; one retry clears it
        import time
        time.sleep(2.0)
        res = run_bass_kernel_spmd(nc, in_maps, core_ids=list(range(N_CORES)))

    # gather: each core returns rm [128, N] bf16 = min over its bank tiles of
    # (m_sq - 2 x.m), per (bank slot, query). Min over slots and cores, then
    # + x_sq and sqrt.
    per_core = np.stack([
        np.asarray(res.results[c]["rm_out"], dtype=np.float32).min(axis=0)
        for c in range(N_CORES)
    ])
    dist_sq = np.maximum(per_core.min(axis=0) + x_sq, 0.0)
    return np.sqrt(dist_sq).reshape(N, 1).astype(np.float32)
